# revision 1
# baseline (speedup 1.0000x reference)
"""CTC loss (nn.CTCLoss, blank=0, reduction='mean', zero_infinity=True) for
T=160, B=64, C=6625, S=25 on 8 TRN2 NeuronCores.

Sharding: data-parallel over batch — 8 of the 64 samples per core.

Algorithm (mathematically identical to the log-domain reference): the CTC
forward DP runs in the probability domain with periodic rescaling.  With
p[t,s] = exp(score of extended-target symbol s at time t) and
q = p * skip_mask, each step is

    alpha_new[s] = alpha[s-2]*q[t,s] + alpha[s-1]*p[t,s] + alpha[s]*p[t,s]

computed as TWO Vector-engine ops on an [8, 51, 3] tile: one elementwise
multiply of the overlapped 3-tap view of alpha against a pre-interleaved
(q,p,p) coefficient array, then a strided reduce_sum over the tap axis.
Every 8 steps the per-sample sum is folded out as log(scale).  Only the 51
extended-target class columns are gathered from the predictions shard
(indirect DMA); the other 6574 classes are never read.
"""

import numpy as np

import concourse.bacc as bacc
import concourse.bass as bass
import concourse.mybir as mybir
import concourse.tile as tile
from concourse.bass_utils import run_bass_kernel_spmd

T = 160
B = 64
C = 6625
S = 25
L = 2 * S + 1  # 51
NCORES = 8
BLOC = B // NCORES  # 8 samples per core
NORM_EVERY = 8
NG = (BLOC * L + 127) // 128  # 4 gather blocks of 128 rows (408 pad to 512)

F32 = mybir.dt.float32
I32 = mybir.dt.int32
ALU = mybir.AluOpType
ACTF = mybir.ActivationFunctionType
AXIS = mybir.AxisListType


def _combined_act_tables(module_arch):
    """Force Exp and Ln into one table set (one ~2.7us ACT_TABLE_LOAD instead
    of two).  Set names/positions are preserved (ids are positional); sets
    other than the combined exp+ln one just stop advertising Exp/Ln."""
    tables = dict(_orig_get_activation_tables(module_arch))
    both = {ACTF.Exp, ACTF.Ln}
    combined = [n for n, fns in tables.items() if both <= fns]
    if combined:
        keep = combined[0]
        for n in list(tables):
            if n != keep and (tables[n] & both):
                tables[n] = tables[n] - both
    return tables


_orig_get_activation_tables = bacc.get_activation_tables


def build_nc(loop_T: int = T) -> bass.Bass:
    bacc.get_activation_tables = _combined_act_tables
    nc = bacc.Bacc("TRN2", target_bir_lowering=False)

    preds = nc.dram_tensor("preds", [BLOC * C, T], F32, kind="ExternalInput")
    gidx_d = nc.dram_tensor("gidx", [128, NG], I32, kind="ExternalInput")
    maskc_d = nc.dram_tensor("maskcol", [128, NG], F32, kind="ExternalInput")
    oneh_d = nc.dram_tensor("onehot", [BLOC, L], F32, kind="ExternalInput")
    out_d = nc.dram_tensor("out2", [BLOC, 2], F32, kind="ExternalOutput")
    pscr_p = nc.dram_tensor("pscr_p", [128 * NG, T], F32)  # internal bounce
    pscr_q = nc.dram_tensor("pscr_q", [128 * NG, T], F32)

    n_scales = (T - 2) // NORM_EVERY  # t=7,15,...,151 -> 19 rescales
    with tile.TileContext(nc) as tc:
        with (
            tc.tile_pool(name="big", bufs=1) as bigp,
            tc.tile_pool(name="small", bufs=1) as smallp,
            tc.tile_pool(name="tmp", bufs=2) as tmpp,
        ):
            G = bigp.tile([128, NG, T], F32, tag="G")
            Gp = bigp.tile([128, NG, T], F32, tag="Gp")
            Gq = bigp.tile([128, NG, T], F32, tag="Gq")
            # PPQ[b, l, j, t] = (q, p, p)[j] at (b, l, t); chunked along t so
            # the loop can start as soon as the first chunk's DMAs land.
            TCH = 40
            NCH = (T + TCH - 1) // TCH
            PPQs = [
                bigp.tile([BLOC, L, 3, TCH], F32, tag=f"PPQ{c}", name=f"PPQ{c}")
                for c in range(NCH)
            ]

            gidx = smallp.tile([128, NG], I32, tag="gidx")
            maskc = smallp.tile([128, NG], F32, tag="maskc")
            oneh = smallp.tile([BLOC, L], F32, tag="oneh")
            X = smallp.tile([BLOC, L + 2], F32, tag="X")
            Y = smallp.tile([BLOC, L + 2], F32, tag="Y")
            scales = smallp.tile([BLOC, n_scales + 1], F32, tag="scales")
            logs = smallp.tile([BLOC, n_scales + 1], F32, tag="logs")
            rcol = smallp.tile([BLOC, 1], F32, tag="rcol")
            out_s = smallp.tile([BLOC, 2], F32, tag="out_s")

            nc.sync.dma_start(out=gidx[:, :], in_=gidx_d[:, :])
            nc.sync.dma_start(out=maskc[:, :], in_=maskc_d[:, :])
            nc.sync.dma_start(out=oneh[:, :], in_=oneh_d[:, :])

            # Gather row-per-partition: G[p, j, :] = preds[gidx[p, j], :]
            for j in range(NG):
                nc.gpsimd.indirect_dma_start(
                    out=G[:, j, :],
                    out_offset=None,
                    in_=preds[:, :],
                    in_offset=bass.IndirectOffsetOnAxis(ap=gidx[:, j : j + 1], axis=0),
                )
            # Pipeline exp/mask/bounce per t-chunk so the DP loop can start
            # once chunk 0 lands; chunks 1..3 overlap with the loop.
            # pscr rows are r = j*128 + p  (flat b-major row id b*L + l).
            for c in range(NCH):
                cs = slice(c * TCH, (c + 1) * TCH)
                nc.scalar.activation(Gp[:, :, cs], G[:, :, cs], ACTF.Exp)
                for j in range(NG):
                    # per-partition scalar multiply on the (otherwise idle)
                    # ACT engine, keeping the Vector engine free for the loop
                    nc.scalar.mul(
                        Gq[:, j, cs], Gp[:, j, cs], maskc[:, j : j + 1]
                    )
                out_ap_p = bass.AP(
                    pscr_p, c * TCH, [[T, 128], [128 * T, NG], [1, TCH]]
                )
                out_ap_q = bass.AP(
                    pscr_q, c * TCH, [[T, 128], [128 * T, NG], [1, TCH]]
                )
                nc.sync.dma_start(out=out_ap_p, in_=Gp[:, :, cs])
                nc.sync.dma_start(out=out_ap_q, in_=Gq[:, :, cs])
                in_p = bass.AP(pscr_p, c * TCH, [[L * T, BLOC], [T, L], [1, TCH]])
                in_q = bass.AP(pscr_q, c * TCH, [[L * T, BLOC], [T, L], [1, TCH]])
                nc.sync.dma_start(out=PPQs[c][:, :, 0, :], in_=in_q)
                nc.sync.dma_start(out=PPQs[c][:, :, 1, :], in_=in_p)
                nc.sync.dma_start(out=PPQs[c][:, :, 2, :], in_=in_p)

            # alpha0: [p(0,0), p(0,1), 0, ...] in padded cols 2:4 of X
            nc.vector.memset(X[:, :], 0.0)
            nc.vector.memset(Y[:, :], 0.0)
            nc.vector.tensor_copy(X[:, 2:4], PPQs[0][:, 0:2, 1, 0])

            cur, nxt = X, Y
            apply_norm = False
            for t in range(1, loop_T):
                ppq_t = PPQs[t // TCH][:, :, :, t % TCH]
                xap = cur[:, :]
                xxx = bass.AP(xap.tensor, xap.offset, [xap.ap[0], [1, L], [1, 3]])

                M = tmpp.tile([BLOC, L, 3], F32, tag="M")
                is_norm = t % NORM_EVERY == NORM_EVERY - 1 and t < T - 1
                k = t // NORM_EVERY
                if apply_norm or is_norm:
                    # stt form: optional rescale via scalar, and on norm steps
                    # the accum_out gives sum(M) = sum(alpha_new) for free.
                    # (tensor_tensor_reduce would fuse this cheaper per the
                    # cost model but fails on HW with these overlapped APs.)
                    nc.vector.scalar_tensor_tensor(
                        out=M[:, :, :], in0=xxx,
                        scalar=rcol[:, :] if apply_norm else 1.0, in1=ppq_t,
                        op0=ALU.mult, op1=ALU.mult,
                        accum_out=scales[:, k : k + 1] if is_norm else None,
                    )
                    apply_norm = False
                else:
                    nc.vector.tensor_tensor(
                        out=M[:, :, :], in0=xxx, in1=ppq_t, op=ALU.mult
                    )
                nc.vector.tensor_reduce(
                    out=nxt[:, 2 : L + 2], in_=M[:, :, :], axis=AXIS.X, op=ALU.add
                )
                if is_norm:
                    nc.vector.reciprocal(out=rcol[:, :], in_=scales[:, k : k + 1])
                    apply_norm = True
                cur, nxt = nxt, cur

            # Final-state sum (one more rescale so dot is well-conditioned),
            # then dot = sum_s (alpha[s]/s_fin) * onehot[s].  log(dot) happens
            # on the host: the ACT Ln table clamps inputs below ~1e-20 and dot
            # can be that small; the device only ever Ln's the window sums,
            # which are safely in range.
            nc.vector.tensor_reduce(
                out=scales[:, n_scales : n_scales + 1], in_=cur[:, 2 : L + 2],
                axis=AXIS.X, op=ALU.add,
            )
            nc.vector.reciprocal(out=rcol[:, :], in_=scales[:, n_scales : n_scales + 1])
            z2 = tmpp.tile([BLOC, L], F32, tag="z2")
            nc.vector.scalar_tensor_tensor(
                out=z2[:, :], in0=cur[:, 2 : L + 2], scalar=rcol[:, :], in1=oneh[:, :],
                op0=ALU.mult, op1=ALU.mult,
                accum_out=out_s[:, 1:2],
            )
            # out_s[:, 0] = sum_k log(scale_k) + log(s_fin)
            nc.scalar.activation(logs[:, :], scales[:, :], ACTF.Ln)
            nc.vector.tensor_reduce(
                out=out_s[:, 0:1], in_=logs[:, :], axis=AXIS.X, op=ALU.add
            )
            nc.sync.dma_start(out=out_d[:, :], in_=out_s[:, :])

    try:
        nc.finalize()
    finally:
        bacc.get_activation_tables = _orig_get_activation_tables
    return nc


def host_prep(predictions, targets, target_lengths):
    """Host-side shard + index prep. Returns per-core input maps."""
    predictions = np.asarray(predictions, dtype=np.float32)
    targets = np.asarray(targets)
    target_lengths = np.asarray(target_lengths)

    ext = np.zeros((B, L), dtype=np.int64)
    ext[:, 1::2] = targets
    mask01 = np.zeros((B, L), dtype=np.float32)
    mask01[:, 3::2] = (targets[:, 1:] != targets[:, :-1]).astype(np.float32)
    onehot = np.zeros((B, L), dtype=np.float32)
    idx = (2 * target_lengths).astype(np.int64)
    onehot[np.arange(B), idx] = 1.0
    onehot[np.arange(B), idx - 1] = 1.0

    in_maps = []
    for k in range(NCORES):
        bsl = slice(k * BLOC, (k + 1) * BLOC)
        # [T, BLOC, C] -> [BLOC, C, T] contiguous -> flat [BLOC*C, T]
        pshard = np.ascontiguousarray(
            predictions[:, bsl, :].transpose(1, 2, 0)
        ).reshape(BLOC * C, T)
        gidx = (
            np.arange(BLOC, dtype=np.int64)[:, None] * C + ext[bsl]
        ).astype(np.int32)
        # b-major flat row list, padded to 128*NG, as [128, NG] column-blocks
        gflat = np.zeros(128 * NG, dtype=np.int32)
        gflat[: BLOC * L] = gidx.reshape(-1)
        mflat = np.zeros(128 * NG, dtype=np.float32)
        mflat[: BLOC * L] = mask01[bsl].reshape(-1)
        in_maps.append(
            {
                "preds": pshard,
                "gidx": gflat.reshape(NG, 128).T.copy(),
                "maskcol": mflat.reshape(NG, 128).T.copy(),
                "onehot": onehot[bsl],
            }
        )
    return in_maps


_NC_CACHE = {}


def kernel(predictions, targets, target_lengths):
    if "nc" not in _NC_CACHE:
        _NC_CACHE["nc"] = build_nc()
    nc = _NC_CACHE["nc"]

    in_maps = host_prep(predictions, targets, target_lengths)
    res = run_bass_kernel_spmd(nc, in_maps, core_ids=list(range(NCORES)))
    return finish(res.results, target_lengths)


def finish(results, target_lengths):
    out2 = np.concatenate([r["out2"].reshape(BLOC, 2) for r in results])
    slogsum, dot = out2[:, 0], out2[:, 1]
    with np.errstate(divide="ignore"):
        nll = -(np.log(dot.astype(np.float32)).astype(np.float32) + slogsum)
    lengths = np.asarray(target_lengths).astype(np.float32)
    per = np.where(nll >= 1e29, np.float32(0.0), nll / lengths)
    return np.array(per.mean(), dtype=np.float32)



# revision 2
# speedup vs baseline: 3.7797x; 3.7797x over previous
"""CTC loss (nn.CTCLoss, blank=0, reduction='mean', zero_infinity=True) for
T=160, B=64, C=6625, S=25 on 8 TRN2 NeuronCores.

Sharding: data-parallel over batch - 8 of the 64 samples per core.

Algorithm: the CTC forward DP is lower-triangular in the extended-target
index s (alpha[s] depends only on alpha[s], alpha[s-1], alpha[s-2]), so it
is solved ROW-BY-ROW over s instead of step-by-step over t.  Row s is one
first-order linear recurrence along t,

    alpha_t[s] = (u_t + alpha_{t-1}[s]) * p_t[s],
    u_t        = alpha_{t-1}[s-1] + m[s] * alpha_{t-1}[s-2],

which maps to ONE hardware prefix-scan instruction (tensor_tensor_scan,
op0=add/op1=mult, fp32 internal state) over the whole time axis, preceded
by one scalar_tensor_tensor for the u row (skipped when m[s] is
structurally zero: even s and s=1, i.e. 27 of 51 rows).  The chain over s
is 51 rows x <=2 dependent ops instead of 159 time steps x 2 ops.

T is split in half: a forward chain over t<80 and a mirrored backward
(beta) chain over t>=80 run as two INDEPENDENT dependency chains
interleaved on the Vector engine, hiding each other's issue latency.  The
halves meet in a 51-element dot per sample.

Probabilities stay in fp32 with NO per-step rescaling: the host subtracts
a per-(b,t) centering constant (mean extended-target score + 0.5) from the
scores before upload, which keeps all alpha/beta magnitudes inside fp32
range (empirically |log alpha| < 70); the host adds the centering log-sums
back when assembling the final loss.
"""

import numpy as np

import concourse.bacc as bacc
import concourse.bass as bass
import concourse.mybir as mybir
import concourse.tile as tile
from concourse.bass_utils import run_bass_kernel_spmd

T = 160
B = 64
C = 6625
S = 25
L = 2 * S + 1  # 51
NCORES = 8
BLOC = B // NCORES  # 8 samples per core
T1 = 80  # forward half; backward covers T-T1 = 80
ROWS = L + 2  # 2 leading zero-pad rows (taps s-1, s-2 of rows 0/1)
COLS = T1 + 1  # leading zero-pad column (alpha_{-1})
ODD = [s for s in range(L) if s >= 3 and s % 2 == 1]  # rows needing the u-op
NODD = len(ODD)  # 24
CH = 8  # s-rows per DMA/exp chunk
NCH = (L + CH - 1) // CH  # 7

F32 = mybir.dt.float32
ALU = mybir.AluOpType
ACTF = mybir.ActivationFunctionType
AXIS = mybir.AxisListType


def build_nc() -> bass.Bass:
    nc = bacc.Bacc("TRN2", target_bir_lowering=False)

    scf_d = nc.dram_tensor("scf", [BLOC, L * T1], F32, kind="ExternalInput")
    scb_d = nc.dram_tensor("scb", [BLOC, L * T1], F32, kind="ExternalInput")
    mf_d = nc.dram_tensor("mf", [BLOC, L], F32, kind="ExternalInput")
    mb_d = nc.dram_tensor("mb", [BLOC, L], F32, kind="ExternalInput")
    ib_d = nc.dram_tensor("ib", [BLOC, L], F32, kind="ExternalInput")
    out_d = nc.dram_tensor("tot", [BLOC, 1], F32, kind="ExternalOutput")

    with tile.TileContext(nc) as tc:
        with (
            tc.tile_pool(name="big", bufs=1) as bigp,
            tc.tile_pool(name="small", bufs=1) as smallp,
        ):
            SCF = bigp.tile([BLOC, L, T1], F32, tag="SCF")
            SCB = bigp.tile([BLOC, L, T1], F32, tag="SCB")
            PF = bigp.tile([BLOC, L, T1], F32, tag="PF")
            PB = bigp.tile([BLOC, L, T1], F32, tag="PB")
            AF = bigp.tile([BLOC, ROWS, COLS], F32, tag="AF")
            AB = bigp.tile([BLOC, ROWS, COLS], F32, tag="AB")
            UF = bigp.tile([BLOC, NODD, T1], F32, tag="UF")
            UB = bigp.tile([BLOC, NODD, T1], F32, tag="UB")

            MF = smallp.tile([BLOC, L], F32, tag="MF")
            MB = smallp.tile([BLOC, L], F32, tag="MB")
            IB = smallp.tile([BLOC, L], F32, tag="IB")
            T1A = smallp.tile([BLOC, L], F32, tag="T1A")
            T2A = smallp.tile([BLOC, L], F32, tag="T2A")
            ETA = smallp.tile([BLOC, L], F32, tag="ETA")
            PRD = smallp.tile([BLOC, L], F32, tag="PRD")
            TOT = smallp.tile([BLOC, 1], F32, tag="TOT")

            nc.sync.dma_start(out=MF[:, :], in_=mf_d[:, :])
            nc.sync.dma_start(out=MB[:, :], in_=mb_d[:, :])
            nc.sync.dma_start(out=IB[:, :], in_=ib_d[:, :])

            # Zero only what is read before being written: the two pad rows,
            # the pad column, and column 0 of the u tiles.
            nc.vector.memset(AF[:, 0:2, :], 0.0)
            nc.vector.memset(AB[:, 0:2, :], 0.0)
            nc.vector.memset(AF[:, 2:ROWS, 0], 0.0)
            nc.vector.memset(AB[:, 2:ROWS, 0], 0.0)
            nc.vector.memset(UF[:, :, 0], 0.0)
            nc.vector.memset(UB[:, :, 0], 0.0)

            # Score upload + exp, chunked over s so the row loop can start
            # after the first chunk.
            for c in range(NCH):
                lo = c * CH
                hi = min(L, lo + CH)
                nc.sync.dma_start(
                    out=SCF[:, lo:hi, :], in_=scf_d[:, lo * T1 : hi * T1]
                )
                nc.sync.dma_start(
                    out=SCB[:, lo:hi, :], in_=scb_d[:, lo * T1 : hi * T1]
                )
                nc.scalar.activation(PF[:, lo:hi, :], SCF[:, lo:hi, :], ACTF.Exp)
                nc.scalar.activation(PB[:, lo:hi, :], SCB[:, lo:hi, :], ACTF.Exp)

            # Row loop: forward and backward chains interleaved.
            for s in range(L):
                r = s + 2
                odd = s >= 3 and s % 2 == 1
                if odd:
                    j = (s - 3) // 2
                    nc.vector.scalar_tensor_tensor(
                        out=UF[:, j, 1:T1], in0=AF[:, r - 2, 1:T1],
                        scalar=MF[:, s : s + 1], in1=AF[:, r - 1, 1:T1],
                        op0=ALU.mult, op1=ALU.add,
                    )
                    nc.vector.scalar_tensor_tensor(
                        out=UB[:, j, 1:T1], in0=AB[:, r - 2, 1:T1],
                        scalar=MB[:, s : s + 1], in1=AB[:, r - 1, 1:T1],
                        op0=ALU.mult, op1=ALU.add,
                    )
                    df_ap = UF[:, j, 0:T1]
                    db_ap = UB[:, j, 0:T1]
                else:
                    # u == alpha[s-1] shifted; the pad column supplies u_0 = 0.
                    df_ap = AF[:, r - 1, 0:T1]
                    db_ap = AB[:, r - 1, 0:T1]
                nc.vector.tensor_tensor_scan(
                    out=AF[:, r, 1:COLS], data0=df_ap, data1=PF[:, s, :],
                    initial=1.0 if s < 2 else 0.0,
                    op0=ALU.add, op1=ALU.mult,
                )
                nc.vector.tensor_tensor_scan(
                    out=AB[:, r, 1:COLS], data0=db_ap, data1=PB[:, s, :],
                    initial=IB[:, s : s + 1],
                    op0=ALU.add, op1=ALU.mult,
                )

            # Combine halves: eta[r] = beta[r] + beta[r-1] + mb[r]*beta[r-2]
            # at the boundary column, then dot with the forward column
            # (s ascending == r descending).
            nc.vector.tensor_tensor(
                out=T1A[:, :], in0=AB[:, 2:ROWS, T1], in1=AB[:, 1 : ROWS - 1, T1],
                op=ALU.add,
            )
            nc.vector.tensor_tensor(
                out=T2A[:, :], in0=MB[:, :], in1=AB[:, 0 : ROWS - 2, T1],
                op=ALU.mult,
            )
            nc.vector.tensor_tensor(
                out=ETA[:, :], in0=T1A[:, :], in1=T2A[:, :], op=ALU.add
            )
            nc.vector.tensor_tensor(
                out=PRD[:, :], in0=AF[:, 2:ROWS, T1], in1=ETA[:, ::-1],
                op=ALU.mult,
            )
            nc.vector.tensor_reduce(
                out=TOT[:, :], in_=PRD[:, :], axis=AXIS.X, op=ALU.add
            )
            nc.sync.dma_start(out=out_d[:, :], in_=TOT[:, :])

    nc.finalize()
    return nc


def host_prep(predictions, targets, target_lengths):
    """Shard + index prep: gather extended-target score rows, center them,
    and lay out forward/reversed-backward halves per core."""
    predictions = np.asarray(predictions, dtype=np.float32)
    targets = np.asarray(targets)
    target_lengths = np.asarray(target_lengths)

    ext = np.zeros((B, L), dtype=np.int64)
    ext[:, 1::2] = targets
    m = np.zeros((B, L), dtype=np.float32)
    m[:, 3::2] = (targets[:, 1:] != targets[:, :-1]).astype(np.float32)
    accept = np.zeros((B, L), dtype=np.float32)
    idx = (2 * target_lengths).astype(np.int64)
    accept[np.arange(B), idx] = 1.0
    accept[np.arange(B), idx - 1] = 1.0

    # g[b, s, t] = predictions[t, b, ext[b, s]]
    g = predictions[:, np.arange(B)[:, None], ext]  # [T, B, L]
    g = np.ascontiguousarray(g.transpose(1, 2, 0))  # [B, L, T]
    logc = g.mean(axis=1) + np.float32(0.5)  # [B, T]
    gc = g - logc[:, None, :]

    # backward: rows r <-> s = 50-r, time reversed; mask mb[r] = m[52-r]
    gb = gc[:, ::-1, ::-1][:, :, :T - T1]
    mb = np.zeros((B, L), dtype=np.float32)
    mb[:, 2:] = m[:, 2:][:, ::-1]  # mb[r] = m[52-r], r in [2, 50]
    ib = accept[:, ::-1]  # ib[r] = accept[50-r]

    in_maps = []
    for k in range(NCORES):
        bsl = slice(k * BLOC, (k + 1) * BLOC)
        in_maps.append(
            {
                "scf": np.ascontiguousarray(
                    gc[bsl, :, :T1].reshape(BLOC, L * T1)
                ),
                "scb": np.ascontiguousarray(gb[bsl].reshape(BLOC, L * T1)),
                "mf": np.ascontiguousarray(m[bsl]),
                "mb": np.ascontiguousarray(mb[bsl]),
                "ib": np.ascontiguousarray(ib[bsl]),
            }
        )
    return in_maps, logc


_NC_CACHE = {}


def kernel(predictions, targets, target_lengths):
    if "nc" not in _NC_CACHE:
        _NC_CACHE["nc"] = build_nc()
    nc = _NC_CACHE["nc"]

    in_maps, logc = host_prep(predictions, targets, target_lengths)
    res = run_bass_kernel_spmd(nc, in_maps, core_ids=list(range(NCORES)))
    return finish(res.results, target_lengths, logc)


def finish(results, target_lengths, logc):
    tot = np.concatenate([r["tot"].reshape(BLOC) for r in results])
    with np.errstate(divide="ignore"):
        nll = -(np.log(tot.astype(np.float64)) + logc.astype(np.float64).sum(axis=1))
    lengths = np.asarray(target_lengths).astype(np.float64)
    per = np.where(~np.isfinite(nll) | (nll >= 1e29), 0.0, nll / lengths)
    return np.array(per.mean(), dtype=np.float32)


# revision 6
# speedup vs baseline: 4.8279x; 1.2773x over previous
"""CTC loss (nn.CTCLoss, blank=0, reduction='mean', zero_infinity=True) for
T=160, B=64, C=6625, S=25 on 8 TRN2 NeuronCores.

Sharding: data-parallel over batch - 8 of the 64 samples per core.

Algorithm: the CTC forward DP is lower-triangular in the extended-target
index s (alpha[s] depends only on alpha[s], alpha[s-1], alpha[s-2]), so it
is solved ROW-BY-ROW over s instead of step-by-step over t.  Row s is one
first-order linear recurrence along t,

    alpha_t[s] = (u_t + alpha_{t-1}[s]) * p_t[s],
    u_t        = alpha_{t-1}[s-1] + m[s] * alpha_{t-1}[s-2],

i.e. ONE hardware prefix-scan (tensor_tensor_scan, op0=add/op1=mult, fp32
state) over the whole time axis, preceded by one scalar_tensor_tensor for
the u row - and u degenerates to the raw alpha[s-1] row (pure AP shift into
the zero pad column) whenever m[s] is structurally zero: even s and s=1,
i.e. 27 of the 51 rows need no u op at all.

T is split in half: a forward chain over t<80 and a mirrored backward
(beta) chain over t>=80.  Because the scan is per-partition independent
and its cost depends only on free-axis length, BOTH chains ride in the
SAME instructions: partitions 0-7 carry the 8 forward samples, partitions
8-15 the backward ones (mask/init/score data selects the direction), so
the whole DP is 51 scans + 24 stts of [16, 80].  The device returns the
two boundary columns (alpha_79 / beta-hat_80, 53 values per sample); the
host contracts them (a 51-element masked dot per sample) while assembling
the final loss - the same finishing step that already applies the log and
the batch mean.

Probabilities stay in fp32 with NO per-step rescaling: the host subtracts
a per-(b,t) centering constant (mean extended-target score + 0.5) from the
scores before upload, which keeps all alpha/beta magnitudes inside fp32
range (empirically |log alpha| < 70); the host adds the centering log-sums
back when assembling the final loss.
"""

import numpy as np

import concourse.bacc as bacc
import concourse.bass as bass
import concourse.mybir as mybir
import concourse.tile as tile
from concourse.bass_utils import run_bass_kernel_spmd

T = 160
B = 64
C = 6625
S = 25
L = 2 * S + 1  # 51
NCORES = 8
BLOC = B // NCORES  # 8 samples per core
T1 = 80  # forward half; backward covers T-T1 = 80
NP = 2 * BLOC  # 16 partitions: 0-7 forward, 8-15 backward
ROWS = L + 2  # 2 leading zero-pad rows (taps s-1, s-2 of rows 0/1)
COLS = T1 + 1  # leading zero-pad column (alpha_{-1})
NODD = 24  # odd rows s=3,5,...,49 need the u-op
SM = 2 * L  # leading ini|msk columns of the staged input
CHUNKS = [2, 2, 4, 8, 8, 8, 8, 8, 3]  # s-rows per DMA+exp chunk (small first)

F32 = mybir.dt.float32
ALU = mybir.AluOpType
ACTF = mybir.ActivationFunctionType


def build_nc() -> bass.Bass:
    nc = bacc.Bacc("TRN2", target_bir_lowering=False)

    # cols [0:L) ini | [L:2L) msk | [2L:) scores, s-major, T1 per row
    sc_d = nc.dram_tensor("sc", [NP, SM + L * T1], F32, kind="ExternalInput")
    col_d = nc.dram_tensor("col", [NP, ROWS], F32, kind="ExternalOutput")

    with tile.TileContext(nc) as tc:
        with (
            tc.tile_pool(name="big", bufs=1) as bigp,
            tc.tile_pool(name="small", bufs=1) as smallp,
        ):
            SC = bigp.tile([NP, SM + L * T1], F32, tag="SC")
            P = bigp.tile([NP, L * T1], F32, tag="P")
            A = bigp.tile([NP, ROWS, COLS], F32, tag="A")
            U = bigp.tile([NP, NODD, T1], F32, tag="U")
            XC = smallp.tile([NP, ROWS], F32, tag="XC")

            # Pad zeroing on the (idle until the first scan) Vector engine.
            nc.vector.memset(A[:, 0:2, :], 0.0)
            nc.vector.memset(A[:, 2:ROWS, 0], 0.0)
            nc.vector.memset(U[:, :, 0], 0.0)

            # Score upload + exp, chunked over s so the row loop starts
            # after the first (small) chunk, which also carries ini|msk.
            lo = 0
            for c, ch in enumerate(CHUNKS):
                hi = lo + ch
                st = 0 if c == 0 else SM + lo * T1
                nc.sync.dma_start(
                    out=SC[:, st : SM + hi * T1], in_=sc_d[:, st : SM + hi * T1]
                )
                nc.scalar.activation(
                    P[:, lo * T1 : hi * T1],
                    SC[:, SM + lo * T1 : SM + hi * T1],
                    ACTF.Exp,
                )
                lo = hi

            # Row loop: both chains fused, one scan (+ one stt) per row.
            for s in range(L):
                r = s + 2
                if s >= 3 and s % 2 == 1:
                    j = (s - 3) // 2
                    nc.vector.scalar_tensor_tensor(
                        out=U[:, j, 1:T1], in0=A[:, r - 2, 1:T1],
                        scalar=SC[:, L + s : L + s + 1], in1=A[:, r - 1, 1:T1],
                        op0=ALU.mult, op1=ALU.add,
                    )
                    d_ap = U[:, j, 0:T1]
                else:
                    # u == alpha[s-1] shifted; the pad column supplies u_0 = 0.
                    d_ap = A[:, r - 1, 0:T1]
                nc.vector.tensor_tensor_scan(
                    out=A[:, r, 1:COLS], data0=d_ap,
                    data1=P[:, s * T1 : (s + 1) * T1],
                    initial=SC[:, s : s + 1],
                    op0=ALU.add, op1=ALU.mult,
                )

            # Boundary columns out (contiguous copy, then one plain DMA).
            nc.vector.tensor_copy(XC[:, :], A[:, 0:ROWS, T1])
            nc.sync.dma_start(out=col_d[:, :], in_=XC[:, :])

    nc.finalize()
    return nc


def host_prep(predictions, targets, target_lengths):
    """Shard + index prep: gather extended-target score rows, center them,
    and pack forward (partitions 0-7) and reversed-backward (8-15) halves."""
    predictions = np.asarray(predictions, dtype=np.float32)
    targets = np.asarray(targets)
    target_lengths = np.asarray(target_lengths)

    ext = np.zeros((B, L), dtype=np.int64)
    ext[:, 1::2] = targets
    m = np.zeros((B, L), dtype=np.float32)
    m[:, 3::2] = (targets[:, 1:] != targets[:, :-1]).astype(np.float32)
    accept = np.zeros((B, L), dtype=np.float32)
    idx = (2 * target_lengths).astype(np.int64)
    accept[np.arange(B), idx] = 1.0
    accept[np.arange(B), idx - 1] = 1.0

    # g[b, s, t] = predictions[t, b, ext[b, s]]
    g = predictions[:, np.arange(B)[:, None], ext]  # [T, B, L]
    g = np.ascontiguousarray(g.transpose(1, 2, 0))  # [B, L, T]
    logc = g.mean(axis=1) + np.float32(0.5)  # [B, T]
    gc = g - logc[:, None, :]

    # backward: rows r <-> s = 50-r, time reversed; mask mb[r] = m[52-r]
    gb = gc[:, ::-1, ::-1][:, :, : T - T1]
    mb = np.zeros((B, L), dtype=np.float32)
    mb[:, 2:] = m[:, 2:][:, ::-1]  # mb[r] = m[52-r], r in [2, 50]
    ib = accept[:, ::-1]  # ib[r] = accept[50-r]
    inif = np.zeros((B, L), dtype=np.float32)
    inif[:, 0:2] = 1.0

    in_maps = []
    for k in range(NCORES):
        bsl = slice(k * BLOC, (k + 1) * BLOC)
        sc = np.concatenate(
            [
                np.concatenate([inif[bsl], m[bsl]], axis=1),
                gc[bsl, :, :T1].reshape(BLOC, L * T1),
            ],
            axis=1,
        )
        sb = np.concatenate(
            [
                np.concatenate([ib[bsl], mb[bsl]], axis=1),
                gb[bsl].reshape(BLOC, L * T1),
            ],
            axis=1,
        )
        in_maps.append({"sc": np.ascontiguousarray(np.concatenate([sc, sb]))})
    return in_maps, (logc, mb)


_NC_CACHE = {}


def kernel(predictions, targets, target_lengths):
    if "nc" not in _NC_CACHE:
        _NC_CACHE["nc"] = build_nc()
    nc = _NC_CACHE["nc"]

    in_maps, aux = host_prep(predictions, targets, target_lengths)
    res = run_bass_kernel_spmd(nc, in_maps, core_ids=list(range(NCORES)))
    return finish(res.results, target_lengths, aux)


def finish(results, target_lengths, aux):
    logc, mb = aux
    cols = np.concatenate(
        [r["col"].reshape(NP, ROWS) for r in results]
    ).astype(np.float64)  # per core: rows 0:8 fwd col, 8:16 bwd col
    af = np.concatenate([cols[k * NP : k * NP + BLOC, 2:] for k in range(NCORES)])
    db = np.concatenate(
        [cols[k * NP + BLOC : (k + 1) * NP, :] for k in range(NCORES)]
    )
    # eta[r] = beta[r] + beta[r-1] + mb[r]*beta[r-2]; dot reverses r vs s
    eta = db[:, 2:] + db[:, 1:-1] + mb * db[:, :-2]
    tot = (af * eta[:, ::-1]).sum(axis=1)
    with np.errstate(divide="ignore", invalid="ignore"):
        nll = -(np.log(tot) + logc.astype(np.float64).sum(axis=1))
    lengths = np.asarray(target_lengths).astype(np.float64)
    per = np.where(~np.isfinite(nll) | (nll >= 1e29), 0.0, nll / lengths)
    return np.array(per.mean(), dtype=np.float32)


# revision 15
# speedup vs baseline: 5.3862x; 1.1156x over previous
"""CTC loss (nn.CTCLoss, blank=0, reduction='mean', zero_infinity=True) for
T=160, B=64, C=6625, S=25 on 8 TRN2 NeuronCores.

Sharding: data-parallel over batch - 8 of the 64 samples per core.

Algorithm: the CTC forward DP is lower-triangular in the extended-target
index s (alpha[s] depends only on alpha[s], alpha[s-1], alpha[s-2]), so it
is solved ROW-BY-ROW over s instead of step-by-step over t.  Row s is one
first-order linear recurrence along t,

    alpha_t[s] = (u_t + alpha_{t-1}[s]) * p_t[s],
    u_t        = alpha_{t-1}[s-1] + m[s] * alpha_{t-1}[s-2],

i.e. ONE hardware prefix-scan (tensor_tensor_scan, op0=add/op1=mult, fp32
state) over the whole time axis, preceded by one scalar_tensor_tensor for
the u row - and u degenerates to the raw alpha[s-1] row (pure AP shift into
the zero pad column) whenever m[s] is structurally zero: even s and s=1,
i.e. 27 of the 51 rows need no u op at all.

T is split in half: a forward chain over t<80 and a mirrored backward
(beta) chain over t>=80.  Because the scan is per-partition independent
and its cost depends only on free-axis length, BOTH chains ride in the
SAME instructions: partitions 0-7 carry the 8 forward samples, partitions
8-15 the backward ones (mask/init/score data selects the direction), so
the whole DP is 51 scans + 24 stts of [16, 80].  The device returns the
two boundary columns (alpha_79 / beta-hat_80, 53 values per sample); the
host contracts them (a 51-element masked dot per sample) while assembling
the final loss - the same finishing step that already applies the log and
the batch mean.

Probabilities stay in fp32 with NO per-step rescaling: the host subtracts
a per-(b,t) centering constant (mean extended-target score + 0.5) from the
scores before upload, which keeps all alpha/beta magnitudes inside fp32
range (empirically |log alpha| < 70); the host adds the centering log-sums
back when assembling the final loss.
"""

import numpy as np

import concourse.bacc as bacc
import concourse.bass as bass
import concourse.mybir as mybir
import concourse.tile as tile
from concourse.bass_utils import run_bass_kernel_spmd

T = 160
B = 64
C = 6625
S = 25
L = 2 * S + 1  # 51
NCORES = 8
BLOC = B // NCORES  # 8 samples per core
T1 = 80  # forward half; backward covers T-T1 = 80
NP = 2 * BLOC  # 16 partitions: 0-7 forward, 8-15 backward
ROWS = L + 2  # 2 leading zero-pad rows (taps s-1, s-2 of rows 0/1)
COLS = T1 + 1  # leading zero-pad column (alpha_{-1})
NODD = 24  # odd rows s=3,5,...,49 need the u-op
SM = 2 * L  # leading ini|msk columns of the staged input
# s-rows per DMA chunk (chunk 0 also carries ini|msk) and per exp op:
# fine-grained early so the row loop's first iterations aren't starved,
# coarse later where the pipeline is far ahead.
CHUNKS = [4, 8, 8, 8, 8, 8, 7]
EXPS = [2, 2, 8, 8, 8, 8, 8, 7]  # padded/truncated to 51 rows
STT_POOL = 0  # 0: stts on DVE; 1: pass-2 stts on gpsimd; 2: all on gpsimd
TH = T1 // 2  # column split: the scans run as two 40-col passes, pass 2
LAG = 2  # trailing pass 1 by LAG rows as an independent dependency chain

F32 = mybir.dt.float32
ALU = mybir.AluOpType
ACTF = mybir.ActivationFunctionType


def build_nc() -> bass.Bass:
    nc = bacc.Bacc("TRN2", target_bir_lowering=False)

    # cols [0:L) ini | [L:2L) msk | [2L:) scores, s-major, T1 per row
    sc_d = nc.dram_tensor("sc", [NP, SM + L * T1], F32, kind="ExternalInput")
    col_d = nc.dram_tensor("col", [NP, ROWS], F32, kind="ExternalOutput")

    with tile.TileContext(nc) as tc:
        with (
            tc.tile_pool(name="big", bufs=1) as bigp,
            tc.tile_pool(name="small", bufs=1) as smallp,
        ):
            SC = bigp.tile([NP, SM + L * T1], F32, tag="SC")
            P = bigp.tile([NP, L * T1], F32, tag="P")
            A = bigp.tile([NP, ROWS, COLS], F32, tag="A")
            U = bigp.tile([NP, NODD, T1], F32, tag="U")
            XC = smallp.tile([NP, ROWS], F32, tag="XC")

            # Pad zeroing on the (idle until the first scan) Vector engine.
            nc.vector.memset(A[:, 0:2, :], 0.0)
            nc.vector.memset(A[:, 2:ROWS, 0], 0.0)
            nc.vector.memset(U[:, :, 0], 0.0)

            # Score upload + exp, chunked over s so the row loop starts
            # after the first (small) chunk, which also carries ini|msk.
            # exp boundaries: prefix sums of EXPS, clipped to [0, L]
            ebnd = [0]
            for e in EXPS:
                if ebnd[-1] >= L:
                    break
                ebnd.append(min(L, ebnd[-1] + e))
            while ebnd[-1] < L:
                ebnd.append(min(L, ebnd[-1] + EXPS[-1]))

            lo = 0
            done_exp = 0
            for c, ch in enumerate(CHUNKS):
                hi = lo + ch
                st = 0 if c == 0 else SM + lo * T1
                nc.sync.dma_start(
                    out=SC[:, st : SM + hi * T1], in_=sc_d[:, st : SM + hi * T1]
                )
                # emit every exp op fully covered by the DMAs issued so far
                while done_exp + 1 < len(ebnd) and ebnd[done_exp + 1] <= hi:
                    elo, ehi = ebnd[done_exp], ebnd[done_exp + 1]
                    nc.scalar.activation(
                        P[:, elo * T1 : ehi * T1],
                        SC[:, SM + elo * T1 : SM + ehi * T1],
                        ACTF.Exp,
                    )
                    done_exp += 1
                lo = hi

            # Row loop: both directions fused per instruction; each row is a
            # stt + scan over cols [0, TH) (pass 1) and, LAG rows behind, the
            # same over [TH, T1) (pass 2, seeded by pass 1's last state) -
            # two interleaved dependency chains keep the engine busy through
            # the per-op semaphore/ack latency.
            def row_pass(s, p):
                r = s + 2
                clo, chi = (0, TH) if p == 0 else (TH, T1)
                if s >= 3 and s % 2 == 1:
                    j = (s - 3) // 2
                    eng = (
                        nc.gpsimd
                        if STT_POOL == 2 or (STT_POOL == 1 and p == 1)
                        else nc.vector
                    )
                    eng.scalar_tensor_tensor(
                        out=U[:, j, max(clo, 1) : chi],
                        in0=A[:, r - 2, max(clo, 1) : chi],
                        scalar=SC[:, L + s : L + s + 1],
                        in1=A[:, r - 1, max(clo, 1) : chi],
                        op0=ALU.mult, op1=ALU.add,
                    )
                    d_ap = U[:, j, clo:chi]
                else:
                    # u == alpha[s-1] shifted; pad column supplies u_0 = 0.
                    d_ap = A[:, r - 1, clo:chi]
                nc.vector.tensor_tensor_scan(
                    out=A[:, r, clo + 1 : chi + 1], data0=d_ap,
                    data1=P[:, s * T1 + clo : s * T1 + chi],
                    initial=SC[:, s : s + 1] if p == 0 else A[:, r, TH : TH + 1],
                    op0=ALU.add, op1=ALU.mult,
                )

            for s in range(L):
                row_pass(s, 0)
                if s >= LAG:
                    row_pass(s - LAG, 1)
            for s in range(L - LAG, L):
                row_pass(s, 1)

            # Boundary columns out (contiguous copy, then one plain DMA).
            nc.vector.tensor_copy(XC[:, :], A[:, 0:ROWS, T1])
            nc.sync.dma_start(out=col_d[:, :], in_=XC[:, :])

    nc.finalize()
    return nc


def host_prep(predictions, targets, target_lengths):
    """Shard + index prep: gather extended-target score rows, center them,
    and pack forward (partitions 0-7) and reversed-backward (8-15) halves."""
    predictions = np.asarray(predictions, dtype=np.float32)
    targets = np.asarray(targets)
    target_lengths = np.asarray(target_lengths)

    ext = np.zeros((B, L), dtype=np.int64)
    ext[:, 1::2] = targets
    m = np.zeros((B, L), dtype=np.float32)
    m[:, 3::2] = (targets[:, 1:] != targets[:, :-1]).astype(np.float32)
    accept = np.zeros((B, L), dtype=np.float32)
    idx = (2 * target_lengths).astype(np.int64)
    accept[np.arange(B), idx] = 1.0
    accept[np.arange(B), idx - 1] = 1.0

    # g[b, s, t] = predictions[t, b, ext[b, s]]
    g = predictions[:, np.arange(B)[:, None], ext]  # [T, B, L]
    g = np.ascontiguousarray(g.transpose(1, 2, 0))  # [B, L, T]
    logc = g.mean(axis=1) + np.float32(0.5)  # [B, T]
    gc = g - logc[:, None, :]

    # backward: rows r <-> s = 50-r, time reversed; mask mb[r] = m[52-r]
    gb = gc[:, ::-1, ::-1][:, :, : T - T1]
    mb = np.zeros((B, L), dtype=np.float32)
    mb[:, 2:] = m[:, 2:][:, ::-1]  # mb[r] = m[52-r], r in [2, 50]
    ib = accept[:, ::-1]  # ib[r] = accept[50-r]
    inif = np.zeros((B, L), dtype=np.float32)
    inif[:, 0:2] = 1.0

    in_maps = []
    for k in range(NCORES):
        bsl = slice(k * BLOC, (k + 1) * BLOC)
        sc = np.concatenate(
            [
                np.concatenate([inif[bsl], m[bsl]], axis=1),
                gc[bsl, :, :T1].reshape(BLOC, L * T1),
            ],
            axis=1,
        )
        sb = np.concatenate(
            [
                np.concatenate([ib[bsl], mb[bsl]], axis=1),
                gb[bsl].reshape(BLOC, L * T1),
            ],
            axis=1,
        )
        in_maps.append({"sc": np.ascontiguousarray(np.concatenate([sc, sb]))})
    return in_maps, (logc, mb)


_NC_CACHE = {}


def kernel(predictions, targets, target_lengths):
    if "nc" not in _NC_CACHE:
        _NC_CACHE["nc"] = build_nc()
    nc = _NC_CACHE["nc"]

    in_maps, aux = host_prep(predictions, targets, target_lengths)
    res = run_bass_kernel_spmd(nc, in_maps, core_ids=list(range(NCORES)))
    return finish(res.results, target_lengths, aux)


def finish(results, target_lengths, aux):
    logc, mb = aux
    cols = np.concatenate(
        [r["col"].reshape(NP, ROWS) for r in results]
    ).astype(np.float64)  # per core: rows 0:8 fwd col, 8:16 bwd col
    af = np.concatenate([cols[k * NP : k * NP + BLOC, 2:] for k in range(NCORES)])
    db = np.concatenate(
        [cols[k * NP + BLOC : (k + 1) * NP, :] for k in range(NCORES)]
    )
    # eta[r] = beta[r] + beta[r-1] + mb[r]*beta[r-2]; dot reverses r vs s
    eta = db[:, 2:] + db[:, 1:-1] + mb * db[:, :-2]
    tot = (af * eta[:, ::-1]).sum(axis=1)
    with np.errstate(divide="ignore", invalid="ignore"):
        nll = -(np.log(tot) + logc.astype(np.float64).sum(axis=1))
    lengths = np.asarray(target_lengths).astype(np.float64)
    per = np.where(~np.isfinite(nll) | (nll >= 1e29), 0.0, nll / lengths)
    return np.array(per.mean(), dtype=np.float32)


# revision 19
# speedup vs baseline: 5.4567x; 1.0131x over previous
"""CTC loss (nn.CTCLoss, blank=0, reduction='mean', zero_infinity=True) for
T=160, B=64, C=6625, S=25 on 8 TRN2 NeuronCores.

Sharding: data-parallel over batch - 8 of the 64 samples per core.

Algorithm: the CTC forward DP is lower-triangular in the extended-target
index s (alpha[s] depends only on alpha[s], alpha[s-1], alpha[s-2]), so it
is solved ROW-BY-ROW over s instead of step-by-step over t.  Row s is one
first-order linear recurrence along t,

    alpha_t[s] = (u_t + alpha_{t-1}[s]) * p_t[s],
    u_t        = alpha_{t-1}[s-1] + m[s] * alpha_{t-1}[s-2],

i.e. ONE hardware prefix-scan (tensor_tensor_scan, op0=add/op1=mult, fp32
state) over the whole time axis, preceded by one scalar_tensor_tensor for
the u row - and u degenerates to the raw alpha[s-1] row (pure AP shift into
the zero pad column) whenever m[s] is structurally zero: even s and s=1,
i.e. 27 of the 51 rows need no u op at all.

T is split in half: a forward chain over t<80 and a mirrored backward
(beta) chain over t>=80.  Because the scan is per-partition independent
and its cost depends only on free-axis length, BOTH chains ride in the
SAME instructions: partitions 0-7 carry the 8 forward samples, partitions
8-15 the backward ones (mask/init/score data selects the direction), so
the whole DP is 51 scans + 24 stts of [16, 80].  The device returns the
two boundary columns (alpha_79 / beta-hat_80, 53 values per sample); the
host contracts them (a 51-element masked dot per sample) while assembling
the final loss - the same finishing step that already applies the log and
the batch mean.

Probabilities stay in fp32 with NO per-step rescaling: the host subtracts
a per-(b,t) centering constant (mean extended-target score + 0.5) from the
scores before upload, which keeps all alpha/beta magnitudes inside fp32
range (empirically |log alpha| < 70); the host adds the centering log-sums
back when assembling the final loss.
"""

import numpy as np

import concourse.bacc as bacc
import concourse.bass as bass
import concourse.mybir as mybir
import concourse.tile as tile
from concourse.bass_utils import run_bass_kernel_spmd

T = 160
B = 64
C = 6625
S = 25
L = 2 * S + 1  # 51
NCORES = 8
BLOC = B // NCORES  # 8 samples per core
T1 = 80  # forward half; backward covers T-T1 = 80
NP = 2 * BLOC  # 16 partitions: 0-7 forward, 8-15 backward
ROWS = L + 2  # 2 leading zero-pad rows (taps s-1, s-2 of rows 0/1)
COLS = T1 + 1  # leading zero-pad column (alpha_{-1})
NODD = 24  # odd rows s=3,5,...,49 need the u-op
SM = 2 * L  # leading ini|msk columns of the staged input
# s-rows per DMA chunk (chunk 0 also carries ini|msk) and per exp op:
# fine-grained early so the row loop's first iterations aren't starved,
# coarse later where the pipeline is far ahead.
CHUNKS = [4, 8, 12, 27]
EXPS = [2, 2, 8, 12, 27]  # padded/truncated to 51 rows
# NOTE: scalar_tensor_tensor on the Pool/gpsimd engine is rejected by
# neuronx-cc codegen ("Instruction engine check failed (Pool)") - the stts
# must stay on the Vector engine.
STT_POOL = 0  # 0: stts on DVE; 1: pass-2 stts on gpsimd; 2: all on gpsimd
TH = 36  # column split: the scans run as two ~40-col passes, pass 2
LAG = 2  # trailing pass 1 by LAG rows as an independent dependency chain

F32 = mybir.dt.float32
ALU = mybir.AluOpType
ACTF = mybir.ActivationFunctionType


def build_nc() -> bass.Bass:
    nc = bacc.Bacc("TRN2", target_bir_lowering=False)

    # cols [0:L) ini | [L:2L) msk | [2L:) scores, s-major, T1 per row
    sc_d = nc.dram_tensor("sc", [NP, SM + L * T1], F32, kind="ExternalInput")
    col_d = nc.dram_tensor("col", [NP, ROWS], F32, kind="ExternalOutput")

    with tile.TileContext(nc) as tc:
        with (
            tc.tile_pool(name="big", bufs=1) as bigp,
            tc.tile_pool(name="small", bufs=1) as smallp,
        ):
            SC = bigp.tile([NP, SM + L * T1], F32, tag="SC")
            P = bigp.tile([NP, L * T1], F32, tag="P")
            A = bigp.tile([NP, ROWS, COLS], F32, tag="A")
            U = bigp.tile([NP, NODD, T1], F32, tag="U")
            XC = smallp.tile([NP, ROWS], F32, tag="XC")

            # Pad zeroing on the (idle until the first scan) Vector engine.
            nc.vector.memset(A[:, 0:2, :], 0.0)
            nc.vector.memset(A[:, 2:ROWS, 0], 0.0)
            nc.vector.memset(U[:, :, 0], 0.0)

            # Score upload + exp, chunked over s so the row loop starts
            # after the first (small) chunk, which also carries ini|msk.
            # exp boundaries: prefix sums of EXPS, clipped to [0, L]
            ebnd = [0]
            for e in EXPS:
                if ebnd[-1] >= L:
                    break
                ebnd.append(min(L, ebnd[-1] + e))
            while ebnd[-1] < L:
                ebnd.append(min(L, ebnd[-1] + EXPS[-1]))

            lo = 0
            done_exp = 0
            for c, ch in enumerate(CHUNKS):
                hi = lo + ch
                st = 0 if c == 0 else SM + lo * T1
                nc.sync.dma_start(
                    out=SC[:, st : SM + hi * T1], in_=sc_d[:, st : SM + hi * T1]
                )
                # emit every exp op fully covered by the DMAs issued so far
                while done_exp + 1 < len(ebnd) and ebnd[done_exp + 1] <= hi:
                    elo, ehi = ebnd[done_exp], ebnd[done_exp + 1]
                    nc.scalar.activation(
                        P[:, elo * T1 : ehi * T1],
                        SC[:, SM + elo * T1 : SM + ehi * T1],
                        ACTF.Exp,
                    )
                    done_exp += 1
                lo = hi

            # Row loop: both directions fused per instruction; each row is a
            # stt + scan over cols [0, TH) (pass 1) and, LAG rows behind, the
            # same over [TH, T1) (pass 2, seeded by pass 1's last state) -
            # two interleaved dependency chains keep the engine busy through
            # the per-op semaphore/ack latency.
            def row_pass(s, p):
                r = s + 2
                clo, chi = (0, TH) if p == 0 else (TH, T1)
                if s >= 3 and s % 2 == 1:
                    j = (s - 3) // 2
                    eng = (
                        nc.gpsimd
                        if STT_POOL == 2 or (STT_POOL == 1 and p == 1)
                        else nc.vector
                    )
                    eng.scalar_tensor_tensor(
                        out=U[:, j, max(clo, 1) : chi],
                        in0=A[:, r - 2, max(clo, 1) : chi],
                        scalar=SC[:, L + s : L + s + 1],
                        in1=A[:, r - 1, max(clo, 1) : chi],
                        op0=ALU.mult, op1=ALU.add,
                    )
                    d_ap = U[:, j, clo:chi]
                else:
                    # u == alpha[s-1] shifted; pad column supplies u_0 = 0.
                    d_ap = A[:, r - 1, clo:chi]
                nc.vector.tensor_tensor_scan(
                    out=A[:, r, clo + 1 : chi + 1], data0=d_ap,
                    data1=P[:, s * T1 + clo : s * T1 + chi],
                    initial=SC[:, s : s + 1] if p == 0 else A[:, r, TH : TH + 1],
                    op0=ALU.add, op1=ALU.mult,
                )

            for s in range(L):
                row_pass(s, 0)
                if s >= LAG:
                    row_pass(s - LAG, 1)
            for s in range(L - LAG, L):
                row_pass(s, 1)

            # Boundary columns out (contiguous copy, then one plain DMA).
            nc.vector.tensor_copy(XC[:, :], A[:, 0:ROWS, T1])
            nc.sync.dma_start(out=col_d[:, :], in_=XC[:, :])

    nc.finalize()
    return nc


def host_prep(predictions, targets, target_lengths):
    """Shard + index prep: gather extended-target score rows, center them,
    and pack forward (partitions 0-7) and reversed-backward (8-15) halves."""
    predictions = np.asarray(predictions, dtype=np.float32)
    targets = np.asarray(targets)
    target_lengths = np.asarray(target_lengths)

    ext = np.zeros((B, L), dtype=np.int64)
    ext[:, 1::2] = targets
    m = np.zeros((B, L), dtype=np.float32)
    m[:, 3::2] = (targets[:, 1:] != targets[:, :-1]).astype(np.float32)
    accept = np.zeros((B, L), dtype=np.float32)
    idx = (2 * target_lengths).astype(np.int64)
    accept[np.arange(B), idx] = 1.0
    accept[np.arange(B), idx - 1] = 1.0

    # g[b, s, t] = predictions[t, b, ext[b, s]]
    g = predictions[:, np.arange(B)[:, None], ext]  # [T, B, L]
    g = np.ascontiguousarray(g.transpose(1, 2, 0))  # [B, L, T]
    logc = g.mean(axis=1) + np.float32(0.5)  # [B, T]
    gc = g - logc[:, None, :]

    # backward: rows r <-> s = 50-r, time reversed; mask mb[r] = m[52-r]
    gb = gc[:, ::-1, ::-1][:, :, : T - T1]
    mb = np.zeros((B, L), dtype=np.float32)
    mb[:, 2:] = m[:, 2:][:, ::-1]  # mb[r] = m[52-r], r in [2, 50]
    ib = accept[:, ::-1]  # ib[r] = accept[50-r]
    inif = np.zeros((B, L), dtype=np.float32)
    inif[:, 0:2] = 1.0

    in_maps = []
    for k in range(NCORES):
        bsl = slice(k * BLOC, (k + 1) * BLOC)
        sc = np.concatenate(
            [
                np.concatenate([inif[bsl], m[bsl]], axis=1),
                gc[bsl, :, :T1].reshape(BLOC, L * T1),
            ],
            axis=1,
        )
        sb = np.concatenate(
            [
                np.concatenate([ib[bsl], mb[bsl]], axis=1),
                gb[bsl].reshape(BLOC, L * T1),
            ],
            axis=1,
        )
        in_maps.append({"sc": np.ascontiguousarray(np.concatenate([sc, sb]))})
    return in_maps, (logc, mb)


_NC_CACHE = {}


def kernel(predictions, targets, target_lengths):
    if "nc" not in _NC_CACHE:
        _NC_CACHE["nc"] = build_nc()
    nc = _NC_CACHE["nc"]

    in_maps, aux = host_prep(predictions, targets, target_lengths)
    res = run_bass_kernel_spmd(nc, in_maps, core_ids=list(range(NCORES)))
    return finish(res.results, target_lengths, aux)


def finish(results, target_lengths, aux):
    logc, mb = aux
    cols = np.concatenate(
        [r["col"].reshape(NP, ROWS) for r in results]
    ).astype(np.float64)  # per core: rows 0:8 fwd col, 8:16 bwd col
    af = np.concatenate([cols[k * NP : k * NP + BLOC, 2:] for k in range(NCORES)])
    db = np.concatenate(
        [cols[k * NP + BLOC : (k + 1) * NP, :] for k in range(NCORES)]
    )
    # eta[r] = beta[r] + beta[r-1] + mb[r]*beta[r-2]; dot reverses r vs s
    eta = db[:, 2:] + db[:, 1:-1] + mb * db[:, :-2]
    tot = (af * eta[:, ::-1]).sum(axis=1)
    with np.errstate(divide="ignore", invalid="ignore"):
        nll = -(np.log(tot) + logc.astype(np.float64).sum(axis=1))
    lengths = np.asarray(target_lengths).astype(np.float64)
    per = np.where(~np.isfinite(nll) | (nll >= 1e29), 0.0, nll / lengths)
    return np.array(per.mean(), dtype=np.float32)
